# revision 51
# baseline (speedup 1.0000x reference)
"""TRN2 Bass kernel for nn_Attention_m_17815524344494.

Multi-head attention over [B=8, M=4, P=512, H=768], nh=12, hs=64.
Sharding: data-parallel over batch B -> one batch element per NeuronCore (8 cores).

Per-core dataflow (T = M*P = 2048 tokens, all matmuls in float32r ~ TF32):
  1. xT [768,2048] (pre-transposed on host) DMA'd feature-major per modality
  2. qT = Wq^T xT, kT = Wk^T xT (feature-major), v = x Wv (token-major,
     augmented with a ones column per head for free softmax sums)
  3. per (modality, head): scoresT = kT^T q (keys on partitions),
     eT = exp(scoresT/8) via ScalarE, ctxT_unnorm/sums = v_aug^T eT,
     sums broadcast via K=1 outer-product matmul, normalize on VectorE
  4. out = ctxT^T Wo (token-major), DMA to DRAM

Biases are zeros per the problem spec; a numpy fallback handles the
(never exercised) nonzero-bias case.
"""

from contextlib import ExitStack

import numpy as np

import concourse.bass as bass
import concourse.mybir as mybir
from concourse import bacc, bass_utils
from concourse.tile import TileContext
from concourse.masks import make_identity

F32 = mybir.dt.float32
F32R = mybir.dt.float32r
AF = mybir.ActivationFunctionType
ALU = mybir.AluOpType

B, M, PM, H = 8, 4, 512, 768
NH, HS = 12, 64
T = M * PM          # 2048 tokens per core
HC = H // 128       # 6 hidden chunks
TCM = PM // 128     # 4 token chunks per modality


def _emit(tc, ctx):
    nc = tc.nc

    x_ap = nc.dram_tensor("x", [H, T], F32, kind="ExternalInput").ap()
    wq_ap = nc.dram_tensor("wq", [H, H], F32, kind="ExternalInput").ap()
    wk_ap = nc.dram_tensor("wk", [H, H], F32, kind="ExternalInput").ap()
    wv_ap = nc.dram_tensor("wv", [H, H], F32, kind="ExternalInput").ap()
    wo_ap = nc.dram_tensor("wo", [H, H], F32, kind="ExternalInput").ap()
    out_ap = nc.dram_tensor("out", [T, H], F32, kind="ExternalOutput").ap()
    srf_ap = nc.dram_tensor("srf", [M * NH, 512], F32, kind="Internal").ap()

    const = ctx.enter_context(tc.tile_pool(name="const", bufs=1))

    # f32r tiles can't be written by memset/affine_select directly (no
    # f32r rounding on those ISA paths); stage in f32 and copy via DVE.
    onescol = const.tile([128, NH * TCM], F32R)
    with tc.tile_pool(name="stage", bufs=1) as stage:
        ones_stage = stage.tile([128, 64], F32)
        nc.gpsimd.memset(ones_stage[:], 1.0)
        nc.vector.tensor_copy(onescol[:], ones_stage[:, :NH * TCM])

    wpool = ctx.enter_context(tc.tile_pool(name="w", bufs=1))
    xtp = ctx.enter_context(tc.tile_pool(name="xt", bufs=1))
    qpool = ctx.enter_context(tc.tile_pool(name="q", bufs=2))
    kpool = ctx.enter_context(tc.tile_pool(name="k", bufs=2))
    vpool = ctx.enter_context(tc.tile_pool(name="v", bufs=2))
    epool = ctx.enter_context(tc.tile_pool(name="e", bufs=4))
    stpool = ctx.enter_context(tc.tile_pool(name="st", bufs=2))
    bcpool = ctx.enter_context(tc.tile_pool(name="bc", bufs=5))
    cpool = ctx.enter_context(tc.tile_pool(name="ctx", bufs=1))
    opool = ctx.enter_context(tc.tile_pool(name="o", bufs=1))
    ps_big = ctx.enter_context(tc.tile_pool(name="ps_big", bufs=2, space="PSUM"))
    ps_sc = ctx.enter_context(tc.tile_pool(name="ps_sc", bufs=2, space="PSUM"))
    ps_c = ctx.enter_context(tc.tile_pool(name="ps_c", bufs=2, space="PSUM"))

    w_tiles = {}

    def load_weights():
        for name, ap in (("wk", wk_ap), ("wv", wv_ap), ("wo", wo_ap)):
            t = wpool.tile([128, HC, H], F32R, tag=name)
            src = ap.rearrange("(kc p) j -> p kc j", p=128)
            for kc in range(HC):
                nc.gpsimd.dma_start(t[:, kc, :], src[:, kc, :])
            w_tiles[name] = t

    mod = {}

    def emit_load_x(m):
        xt = xtp.tile([128, HC, PM], F32R, tag="xt")
        if m == 0:
            # Interleave x and Wq chunk DMAs so the first projection group's
            # operands land as early as possible, then stream the rest.
            wq = wpool.tile([128, HC, H], F32R, tag="wq", name="wq")
            w_tiles["wq"] = wq
            wq_src = wq_ap.rearrange("(kc p) j -> p kc j", p=128)
            for hc in range(HC):
                nc.gpsimd.dma_start(
                    xt[:, hc, :],
                    x_ap.rearrange("(hc p) t -> p hc t", p=128)[:, hc, :PM],
                )
                nc.gpsimd.dma_start(wq[:, hc, :], wq_src[:, hc, :])
            mod[m] = {"xt": xt}
            load_weights()
            return
        for hc in range(HC):
            nc.gpsimd.dma_start(
                xt[:, hc, :],
                x_ap.rearrange("(hc p) t -> p hc t", p=128)[:, hc, m * PM:(m + 1) * PM],
            )
        mod[m] = {"xt": xt}

    def proj_qk_group(m, which, jc):
        st = mod[m]
        key = "qt" if which == "q" else "kt"
        if key not in st:
            pool = qpool if which == "q" else kpool
            st[key] = pool.tile([128, HC, PM], F32R, tag=which, name=f"{which}t")
        w = w_tiles["wq" if which == "q" else "wk"]
        ps = ps_big.tile([128, 512], F32, tag="ps_big")
        for kc in range(HC):
            nc.tensor.matmul(
                ps[:],
                w[:, kc, jc * 128:(jc + 1) * 128],
                st["xt"][:, kc, :],
                start=(kc == 0),
                stop=(kc == HC - 1),
            )
        nc.vector.tensor_copy(st[key][:, jc, :], ps[:])

    def proj_v_group(m, ti, nn):
        st = mod[m]
        if "vt" not in st:
            st["vt"] = vpool.tile([128, TCM, NH, HS + 1], F32R, tag="v", name="vt")
            nc.vector.tensor_copy(
                st["vt"][:, :, :, HS],
                onescol[:].rearrange("p (t h) -> p t h", t=TCM),
            )
        ps = ps_big.tile([128, 512], F32, tag="ps_big")
        for kc in range(HC):
            nc.tensor.matmul(
                ps[:, :384],
                st["xt"][:, kc, ti * 128:(ti + 1) * 128],
                w_tiles["wv"][:, kc, nn * 384:(nn + 1) * 384],
                start=(kc == 0),
                stop=(kc == HC - 1),
            )
        nc.scalar.activation(
            st["vt"][:, ti, nn * 6:(nn + 1) * 6, :HS],
            ps[:, :384].rearrange("p (h c) -> p h c", c=HS),
            AF.Copy,
        )

    def phase_ab_fillers(m):
        yield lambda: emit_load_x(m)
        for jc in range(HC):
            yield lambda jc=jc: proj_qk_group(m, "q", jc)
        for jc in range(HC):
            yield lambda jc=jc: proj_qk_group(m, "k", jc)
        for ti in range(TCM):
            for nn in range(2):
                yield lambda ti=ti, nn=nn: proj_v_group(m, ti, nn)

    def attention(m, fillers):
        # Per (modality, head): scoresT on PE, exp on ScalarE, PV (with the
        # v_aug ones column producing softmax sums in psum row 64).
        # Normalization is batched (reciprocal_approx_fast + K=1 broadcast
        # matmul + in-place scale) so the PE never waits on the recip chain.
        # Between each head's scores and PV we weave one projection group of
        # the NEXT modality -- independent PE work that fills the exp wait.
        st = mod[m]
        qt, kt, vt = st["qt"], st["kt"], st["vt"]
        ctxt = cpool.tile([128, HC, PM], F32R, tag="ctx")
        st["ctxt"] = ctxt
        rfrs = []
        bcs = []

        def normalize(heads):
            for h in heads:
                hc, hr = h // 2, (h % 2) * 64
                nc.vector.tensor_tensor(
                    ctxt[hr:hr + 64, hc, :], ctxt[hr:hr + 64, hc, :],
                    bcs[h][hr:hr + 64, :], ALU.mult,
                )

        for h in range(NH):
            hc, hr = h // 2, (h % 2) * 64
            qh = qt[hr:hr + 64, hc, :]
            ets = []
            for half in range(2):
                pssc = ps_sc.tile([128, 1024], F32, tag="ps_sc")
                for j2 in range(2):
                    jc = half * 2 + j2
                    nc.tensor.matmul(
                        pssc[:, j2 * 512:(j2 + 1) * 512],
                        kt[hr:hr + 64, hc, jc * 128:(jc + 1) * 128],
                        qh,
                        start=True,
                        stop=True,
                    )
                et = epool.tile([128, 1024], F32R, tag="e")
                nc.scalar.activation(et[:], pssc[:], AF.Exp, scale=0.125)
                ets.append(et)
            npop = 2 if len(fillers) > NH - 1 - h else 1
            for f in fillers[:npop]:
                f()
            del fillers[:npop]
            psc = ps_c.tile([HS + 1, 512], F32, tag="ps_c")
            for jc in range(TCM):
                nc.tensor.matmul(
                    psc[:],
                    vt[:, jc, h, :],
                    ets[jc // 2][:, (jc % 2) * 512:(jc % 2 + 1) * 512],
                    start=(jc == 0),
                    stop=(jc == TCM - 1),
                )
            nc.vector.tensor_copy(ctxt[hr:hr + 64, hc, :], psc[:HS, :])
            stmp = stpool.tile([1, 512], F32, tag="stmp")
            nc.vector.tensor_copy(stmp[:], psc[HS:HS + 1, :])
            rf = stpool.tile([1, 512], F32, tag="rf")
            nc.vector.reciprocal_approx_fast(out=rf[:], in_=stmp[:])
            row = srf_ap[m * NH + h:m * NH + h + 1, :]
            nc.sync.dma_start(row, rf[0:1, :])
            bc = bcpool.tile([128, 512], F32, tag="bc")
            nc.sync.dma_start(bc[:], row.to_broadcast((128, 512)))
            bcs.append(bc)
            rfrs.append(bc)
            if len(rfrs) % 2 == 0:
                normalize(range(len(rfrs) - 2, len(rfrs)))
        for f in fillers:
            f()
        del fillers[:]


    def out_proj(m):
        ctxt = mod[m]["ctxt"]
        for ti in range(TCM):
            osb = opool.tile([128, H], F32, tag="o")
            for nn in range(2):
                ps = ps_big.tile([128, 512], F32, tag="ps_big")
                for cc in range(HC):
                    nc.tensor.matmul(
                        ps[:, :384],
                        ctxt[:, cc, ti * 128:(ti + 1) * 128],
                        w_tiles["wo"][:, cc, nn * 384:(nn + 1) * 384],
                        start=(cc == 0),
                        stop=(cc == HC - 1),
                    )
                nc.scalar.activation(osb[:, nn * 384:(nn + 1) * 384], ps[:, :384], AF.Copy)
            row0 = (m * TCM + ti) * 128
            nc.sync.dma_start(out_ap[row0:row0 + 128, :], osb[:])

    # Modality 0 bootstrap: kc-outer paired projection consumes x/W DMA
    # chunks as they arrive instead of waiting for whole tensors.
    emit_load_x(0)
    for which in ("q", "k"):
        st0 = mod[0]
        key = "qt" if which == "q" else "kt"
        st0[key] = (qpool if which == "q" else kpool).tile(
            [128, HC, PM], F32R, tag=which, name=f"{which}t0")
        w = w_tiles["wq" if which == "q" else "wk"]
        for jcp in range(3):
            psA = ps_big.tile([128, 512], F32, tag="ps_big")
            psB = ps_big.tile([128, 512], F32, tag="ps_big")
            for kc in range(HC):
                nc.tensor.matmul(
                    psA[:], w[:, kc, (2 * jcp) * 128:(2 * jcp + 1) * 128],
                    st0["xt"][:, kc, :], start=(kc == 0), stop=(kc == HC - 1))
                nc.tensor.matmul(
                    psB[:], w[:, kc, (2 * jcp + 1) * 128:(2 * jcp + 2) * 128],
                    st0["xt"][:, kc, :], start=(kc == 0), stop=(kc == HC - 1))
            nc.vector.tensor_copy(st0[key][:, 2 * jcp, :], psA[:])
            nc.vector.tensor_copy(st0[key][:, 2 * jcp + 1, :], psB[:])
    for ti in range(TCM):
        for nn in range(2):
            proj_v_group(0, ti, nn)
    for m in range(M):
        fillers = list(phase_ab_fillers(m + 1)) if m + 1 < M else []
        attention(m, fillers)
        out_proj(m)

_NC_CACHE = {}


def build_nc():
    if "nc" not in _NC_CACHE:
        nc = bacc.Bacc("TRN2", target_bir_lowering=False, debug=False, num_devices=B)
        with TileContext(nc) as tc:
            with ExitStack() as stack:
                _emit(tc, stack)
        nc.compile()
        _NC_CACHE["nc"] = nc
    return _NC_CACHE["nc"]


def _numpy_fallback(x, Wq, bq, Wk, bk, Wv, bv, Wo, bo):
    Bb, Mm, Pp, Hh = x.shape
    xx = x.reshape(-1, Hh)
    q = (xx @ Wq + bq).reshape(Bb, Mm, Pp, NH, HS).transpose(0, 1, 3, 2, 4)
    k = (xx @ Wk + bk).reshape(Bb, Mm, Pp, NH, HS).transpose(0, 1, 3, 2, 4)
    v = (xx @ Wv + bv).reshape(Bb, Mm, Pp, NH, HS).transpose(0, 1, 3, 2, 4)
    s = np.einsum("bmnqh,bmnkh->bmnqk", q, k) / np.sqrt(HS)
    s = s - s.max(axis=-1, keepdims=True)
    e = np.exp(s)
    p = e / e.sum(axis=-1, keepdims=True)
    ctx = np.einsum("bmnqk,bmnkh->bmnqh", p, v)
    ctx = ctx.transpose(0, 1, 3, 2, 4).reshape(Bb, Mm, Pp, Hh)
    return (ctx @ Wo + bo).astype(np.float32)


def kernel(hidden_states, Wq, bq, Wk, bk, Wv, bv, Wo, bo):
    hs = np.ascontiguousarray(np.asarray(hidden_states, dtype=np.float32))
    ws = {n: np.ascontiguousarray(np.asarray(w, dtype=np.float32))
          for n, w in (("wq", Wq), ("wk", Wk), ("wv", Wv), ("wo", Wo))}
    biases = [np.asarray(b, dtype=np.float32) for b in (bq, bk, bv, bo)]
    if any(np.any(b) for b in biases):
        return _numpy_fallback(hs, ws["wq"], biases[0], ws["wk"], biases[1],
                               ws["wv"], biases[2], ws["wo"], biases[3])

    nc = build_nc()
    in_maps = [
        {"x": np.ascontiguousarray(hs[b].reshape(T, H).T), **ws}
        for b in range(B)
    ]
    res = bass_utils.run_bass_kernel_spmd(nc, in_maps, core_ids=list(range(B)))
    out = np.stack([res.results[b]["out"].reshape(M, PM, H) for b in range(B)])
    return out.astype(np.float32)


# revision 53
# speedup vs baseline: 1.0858x; 1.0858x over previous
"""TRN2 Bass kernel for nn_Attention_m_17815524344494.

Multi-head attention over [B=8, M=4, P=512, H=768], nh=12, hs=64.
Sharding: data-parallel over batch B -> one batch element per NeuronCore (8 cores).

Per-core dataflow (T = M*P = 2048 tokens, all matmuls in float32r ~ TF32):
  1. xT [768,2048] (pre-transposed on host) DMA'd feature-major per modality
  2. qT = Wq^T xT, kT = Wk^T xT (feature-major), v = x Wv (token-major,
     augmented with a ones column per head for free softmax sums)
  3. per (modality, head): scoresT = kT^T q (keys on partitions),
     eT = exp(scoresT/8) via ScalarE, ctxT_unnorm/sums = v_aug^T eT,
     sums broadcast via K=1 outer-product matmul, normalize on VectorE
  4. out = ctxT^T Wo (token-major), DMA to DRAM

Biases are zeros per the problem spec; a numpy fallback handles the
(never exercised) nonzero-bias case.
"""

from contextlib import ExitStack

import numpy as np

import concourse.bass as bass
import concourse.mybir as mybir
from concourse import bacc, bass_utils
from concourse.tile import TileContext
from concourse.masks import make_identity

F32 = mybir.dt.float32
F32R = mybir.dt.float32r
AF = mybir.ActivationFunctionType
ALU = mybir.AluOpType

B, M, PM, H = 8, 4, 512, 768
NH, HS = 12, 64
T = M * PM          # 2048 tokens per core
HC = H // 128       # 6 hidden chunks
TCM = PM // 128     # 4 token chunks per modality


def _emit(tc, ctx):
    nc = tc.nc

    x_ap = nc.dram_tensor("x", [H, T], F32, kind="ExternalInput").ap()
    wq_ap = nc.dram_tensor("wq", [H, H], F32, kind="ExternalInput").ap()
    wk_ap = nc.dram_tensor("wk", [H, H], F32, kind="ExternalInput").ap()
    wv_ap = nc.dram_tensor("wv", [H, H], F32, kind="ExternalInput").ap()
    wo_ap = nc.dram_tensor("wo", [H, H], F32, kind="ExternalInput").ap()
    out_ap = nc.dram_tensor("out", [T, H], F32, kind="ExternalOutput").ap()
    srf_ap = nc.dram_tensor("srf", [M * NH, 512], F32, kind="Internal").ap()

    const = ctx.enter_context(tc.tile_pool(name="const", bufs=1))

    # f32r tiles can't be written by memset/affine_select directly (no
    # f32r rounding on those ISA paths); stage in f32 and copy via DVE.
    onescol = const.tile([128, NH * TCM], F32R)
    with tc.tile_pool(name="stage", bufs=1) as stage:
        ones_stage = stage.tile([128, 64], F32)
        nc.gpsimd.memset(ones_stage[:], 1.0)
        nc.vector.tensor_copy(onescol[:], ones_stage[:, :NH * TCM])

    wpool = ctx.enter_context(tc.tile_pool(name="w", bufs=1))
    xtp = ctx.enter_context(tc.tile_pool(name="xt", bufs=1))
    qpool = ctx.enter_context(tc.tile_pool(name="q", bufs=2))
    kpool = ctx.enter_context(tc.tile_pool(name="k", bufs=2))
    vpool = ctx.enter_context(tc.tile_pool(name="v", bufs=2))
    epool = ctx.enter_context(tc.tile_pool(name="e", bufs=8))
    stpool = ctx.enter_context(tc.tile_pool(name="st", bufs=2))
    bcpool = ctx.enter_context(tc.tile_pool(name="bc", bufs=5))
    cpool = ctx.enter_context(tc.tile_pool(name="ctx", bufs=1))
    opool = ctx.enter_context(tc.tile_pool(name="o", bufs=1))
    ps_big = ctx.enter_context(tc.tile_pool(name="ps_big", bufs=2, space="PSUM"))
    ps_sc = ctx.enter_context(tc.tile_pool(name="ps_sc", bufs=4, space="PSUM"))
    ps_c = ctx.enter_context(tc.tile_pool(name="ps_c", bufs=2, space="PSUM"))

    w_tiles = {}

    def load_weights():
        for name, ap in (("wk", wk_ap), ("wv", wv_ap), ("wo", wo_ap)):
            t = wpool.tile([128, HC, H], F32R, tag=name)
            src = ap.rearrange("(kc p) j -> p kc j", p=128)
            for kc in range(HC):
                nc.gpsimd.dma_start(t[:, kc, :], src[:, kc, :])
            w_tiles[name] = t

    mod = {}

    def emit_load_x(m):
        xt = xtp.tile([128, HC, PM], F32R, tag="xt")
        if m == 0:
            # Interleave x and Wq chunk DMAs so the first projection group's
            # operands land as early as possible, then stream the rest.
            wq = wpool.tile([128, HC, H], F32R, tag="wq", name="wq")
            w_tiles["wq"] = wq
            wq_src = wq_ap.rearrange("(kc p) j -> p kc j", p=128)
            for hc in range(HC):
                nc.gpsimd.dma_start(
                    xt[:, hc, :],
                    x_ap.rearrange("(hc p) t -> p hc t", p=128)[:, hc, :PM],
                )
                nc.gpsimd.dma_start(wq[:, hc, :], wq_src[:, hc, :])
            mod[m] = {"xt": xt}
            load_weights()
            return
        for hc in range(HC):
            nc.gpsimd.dma_start(
                xt[:, hc, :],
                x_ap.rearrange("(hc p) t -> p hc t", p=128)[:, hc, m * PM:(m + 1) * PM],
            )
        mod[m] = {"xt": xt}

    def proj_qk_group(m, which, jc):
        st = mod[m]
        key = "qt" if which == "q" else "kt"
        if key not in st:
            pool = qpool if which == "q" else kpool
            st[key] = pool.tile([128, HC, PM], F32R, tag=which, name=f"{which}t")
        w = w_tiles["wq" if which == "q" else "wk"]
        ps = ps_big.tile([128, 512], F32, tag="ps_big")
        for kc in range(HC):
            nc.tensor.matmul(
                ps[:],
                w[:, kc, jc * 128:(jc + 1) * 128],
                st["xt"][:, kc, :],
                start=(kc == 0),
                stop=(kc == HC - 1),
            )
        nc.vector.tensor_copy(st[key][:, jc, :], ps[:])

    def proj_v_group(m, ti, nn):
        st = mod[m]
        if "vt" not in st:
            st["vt"] = vpool.tile([128, TCM, NH, HS + 1], F32R, tag="v", name="vt")
            nc.vector.tensor_copy(
                st["vt"][:, :, :, HS],
                onescol[:].rearrange("p (t h) -> p t h", t=TCM),
            )
        ps = ps_big.tile([128, 512], F32, tag="ps_big")
        for kc in range(HC):
            nc.tensor.matmul(
                ps[:, :384],
                st["xt"][:, kc, ti * 128:(ti + 1) * 128],
                w_tiles["wv"][:, kc, nn * 384:(nn + 1) * 384],
                start=(kc == 0),
                stop=(kc == HC - 1),
            )
        nc.scalar.activation(
            st["vt"][:, ti, nn * 6:(nn + 1) * 6, :HS],
            ps[:, :384].rearrange("p (h c) -> p h c", c=HS),
            AF.Copy,
        )

    def phase_ab_fillers(m):
        yield lambda: emit_load_x(m)
        for jc in range(HC):
            yield lambda jc=jc: proj_qk_group(m, "q", jc)
        for jc in range(HC):
            yield lambda jc=jc: proj_qk_group(m, "k", jc)
        for ti in range(TCM):
            for nn in range(2):
                yield lambda ti=ti, nn=nn: proj_v_group(m, ti, nn)

    def attention(m, fillers):
        # Per (modality, head): scoresT on PE, exp on ScalarE, PV (with the
        # v_aug ones column producing softmax sums in psum row 64).
        # Normalization is batched (reciprocal_approx_fast + K=1 broadcast
        # matmul + in-place scale) so the PE never waits on the recip chain.
        # Between each head's scores and PV we weave one projection group of
        # the NEXT modality -- independent PE work that fills the exp wait.
        st = mod[m]
        qt, kt, vt = st["qt"], st["kt"], st["vt"]
        ctxt = cpool.tile([128, HC, PM], F32R, tag="ctx")
        st["ctxt"] = ctxt
        rfrs = []
        bcs = []

        def normalize(heads):
            for h in heads:
                hc, hr = h // 2, (h % 2) * 64
                nc.vector.tensor_tensor(
                    ctxt[hr:hr + 64, hc, :], ctxt[hr:hr + 64, hc, :],
                    bcs[h][hr:hr + 64, :], ALU.mult,
                )

        for h in range(NH):
            hc, hr = h // 2, (h % 2) * 64
            qh = qt[hr:hr + 64, hc, :]
            ets = []
            for jc in range(TCM):
                pssc = ps_sc.tile([128, 512], F32, tag="ps_sc")
                nc.tensor.matmul(
                    pssc[:],
                    kt[hr:hr + 64, hc, jc * 128:(jc + 1) * 128],
                    qh,
                    start=True,
                    stop=True,
                )
                et = epool.tile([128, 512], F32R, tag="e")
                nc.scalar.activation(et[:], pssc[:], AF.Exp, scale=0.125)
                ets.append(et)
            for f in fillers[:1]:
                f()
            del fillers[:1]
            psc = ps_c.tile([HS + 1, 512], F32, tag="ps_c")
            for jc in range(TCM):
                nc.tensor.matmul(
                    psc[:],
                    vt[:, jc, h, :],
                    ets[jc][:],
                    start=(jc == 0),
                    stop=(jc == TCM - 1),
                )
            nc.vector.tensor_copy(ctxt[hr:hr + 64, hc, :], psc[:HS, :])
            stmp = stpool.tile([1, 512], F32, tag="stmp")
            nc.vector.tensor_copy(stmp[:], psc[HS:HS + 1, :])
            rf = stpool.tile([1, 512], F32, tag="rf")
            nc.vector.reciprocal_approx_fast(out=rf[:], in_=stmp[:])
            row = srf_ap[m * NH + h:m * NH + h + 1, :]
            nc.sync.dma_start(row, rf[0:1, :])
            bc = bcpool.tile([128, 512], F32, tag="bc")
            nc.sync.dma_start(bc[:], row.to_broadcast((128, 512)))
            bcs.append(bc)
            rfrs.append(bc)
            if len(rfrs) in (4, 8):
                normalize(range(len(rfrs) - 4, len(rfrs)))
        for f in fillers:
            f()
        del fillers[:]
        normalize(range(8, NH))


    def out_proj(m):
        ctxt = mod[m]["ctxt"]
        for ti in range(TCM):
            osb = opool.tile([128, H], F32, tag="o")
            for nn in range(2):
                ps = ps_big.tile([128, 512], F32, tag="ps_big")
                for cc in range(HC):
                    nc.tensor.matmul(
                        ps[:, :384],
                        ctxt[:, cc, ti * 128:(ti + 1) * 128],
                        w_tiles["wo"][:, cc, nn * 384:(nn + 1) * 384],
                        start=(cc == 0),
                        stop=(cc == HC - 1),
                    )
                nc.scalar.activation(osb[:, nn * 384:(nn + 1) * 384], ps[:, :384], AF.Copy)
            row0 = (m * TCM + ti) * 128
            nc.sync.dma_start(out_ap[row0:row0 + 128, :], osb[:])

    # Modality 0 bootstrap: kc-outer paired projection consumes x/W DMA
    # chunks as they arrive instead of waiting for whole tensors.
    emit_load_x(0)
    for which in ("q", "k"):
        st0 = mod[0]
        key = "qt" if which == "q" else "kt"
        st0[key] = (qpool if which == "q" else kpool).tile(
            [128, HC, PM], F32R, tag=which, name=f"{which}t0")
        w = w_tiles["wq" if which == "q" else "wk"]
        for jcp in range(3):
            psA = ps_big.tile([128, 512], F32, tag="ps_big")
            psB = ps_big.tile([128, 512], F32, tag="ps_big")
            for kc in range(HC):
                nc.tensor.matmul(
                    psA[:], w[:, kc, (2 * jcp) * 128:(2 * jcp + 1) * 128],
                    st0["xt"][:, kc, :], start=(kc == 0), stop=(kc == HC - 1))
                nc.tensor.matmul(
                    psB[:], w[:, kc, (2 * jcp + 1) * 128:(2 * jcp + 2) * 128],
                    st0["xt"][:, kc, :], start=(kc == 0), stop=(kc == HC - 1))
            nc.vector.tensor_copy(st0[key][:, 2 * jcp, :], psA[:])
            nc.vector.tensor_copy(st0[key][:, 2 * jcp + 1, :], psB[:])
    for ti in range(TCM):
        for nn in range(2):
            proj_v_group(0, ti, nn)
    for m in range(M):
        fillers = list(phase_ab_fillers(m + 1)) if m + 1 < M else []
        attention(m, fillers)
        out_proj(m)

_NC_CACHE = {}


def build_nc():
    if "nc" not in _NC_CACHE:
        nc = bacc.Bacc("TRN2", target_bir_lowering=False, debug=False, num_devices=B)
        with TileContext(nc) as tc:
            with ExitStack() as stack:
                _emit(tc, stack)
        nc.compile()
        _NC_CACHE["nc"] = nc
    return _NC_CACHE["nc"]


def _numpy_fallback(x, Wq, bq, Wk, bk, Wv, bv, Wo, bo):
    Bb, Mm, Pp, Hh = x.shape
    xx = x.reshape(-1, Hh)
    q = (xx @ Wq + bq).reshape(Bb, Mm, Pp, NH, HS).transpose(0, 1, 3, 2, 4)
    k = (xx @ Wk + bk).reshape(Bb, Mm, Pp, NH, HS).transpose(0, 1, 3, 2, 4)
    v = (xx @ Wv + bv).reshape(Bb, Mm, Pp, NH, HS).transpose(0, 1, 3, 2, 4)
    s = np.einsum("bmnqh,bmnkh->bmnqk", q, k) / np.sqrt(HS)
    s = s - s.max(axis=-1, keepdims=True)
    e = np.exp(s)
    p = e / e.sum(axis=-1, keepdims=True)
    ctx = np.einsum("bmnqk,bmnkh->bmnqh", p, v)
    ctx = ctx.transpose(0, 1, 3, 2, 4).reshape(Bb, Mm, Pp, Hh)
    return (ctx @ Wo + bo).astype(np.float32)


def kernel(hidden_states, Wq, bq, Wk, bk, Wv, bv, Wo, bo):
    hs = np.ascontiguousarray(np.asarray(hidden_states, dtype=np.float32))
    ws = {n: np.ascontiguousarray(np.asarray(w, dtype=np.float32))
          for n, w in (("wq", Wq), ("wk", Wk), ("wv", Wv), ("wo", Wo))}
    biases = [np.asarray(b, dtype=np.float32) for b in (bq, bk, bv, bo)]
    if any(np.any(b) for b in biases):
        return _numpy_fallback(hs, ws["wq"], biases[0], ws["wk"], biases[1],
                               ws["wv"], biases[2], ws["wo"], biases[3])

    nc = build_nc()
    in_maps = [
        {"x": np.ascontiguousarray(hs[b].reshape(T, H).T), **ws}
        for b in range(B)
    ]
    res = bass_utils.run_bass_kernel_spmd(nc, in_maps, core_ids=list(range(B)))
    out = np.stack([res.results[b]["out"].reshape(M, PM, H) for b in range(B)])
    return out.astype(np.float32)
